# revision 45
# baseline (speedup 1.0000x reference)
"""Trainium2 Bass kernel for nn_MultiHeadAttention_75737453297867.

Sharding: one head per NeuronCore (8 heads / 8 cores). The reference's
aliased as_strided gather for head h reads the flange-padded k/v storage
at base offset 24576*h = (c_lo planes) + 64*phi elements; since the
aliased view is a pure flat shift, ONE zero-padded slab [4, 144*64] per
buffer serves all three phi variants via gather offsets 3072*v (uniform
stride!). Variant selection happens through host data: zeroed conv
weights for inactive q rows, zeroed wo3 rows for inactive o rows.

Of the 5760 kv per window, 1440 fall on zero-padded columns for every
dph (x mod 64 outside [8,56)): exp(q.k)=1 there, v=0. Only the 4320
valid kv are gathered (5 strided runs, direct compact gather), padded
to 4352=34*128; the denominator gets +1408*e^-A (A=9 shift keeps fp16
exp in range; softmax is shift invariant).

Engines: PE = convs (2-row-stacked x), QK (fp16, out [128,512] = full
PSUM bank), AV (ex stationary, out free=13), transposes. ACT = exp only
(bias=-A, groups of 3 banks double buffered) plus tail copies. DVE =
epilogues/normalize. The final conv is column-split (A: 0:22 window-0
only, B: 22:48, with late rows further split at col 39) and interleaved
one chunk per exp group into attention slack; only 9 narrow col-39:48
pieces remain after the last softmax. PE warm-up matmuls hold the
p-state ramp across the conv-to-attention transition. uk/uv gathers are
split head (kv chunks 0-8) / rest so QK starts as soon as run 0 lands.
"""

import math
import sys

import numpy as np

if "/opt/trn_rl_repo" not in sys.path:
    sys.path.insert(0, "/opt/trn_rl_repo")

import concourse.bass as bass
import concourse.tile as tile
from concourse import bacc
from concourse import mybir
from concourse.bass_types import AP

# Problem constants
CIN, COUT, H, W = 64, 64, 128, 48
DM, NH, DPH = 32, 8, 4
M0, M1 = 144, 40
PHIS = (0, 48, 96)
KVK = 4320                  # kept kv per window
KVP = 4352                  # padded to 34*128
NCH = 34                    # kv chunks of 128
NPER = 36                   # 192-element periods per window
GRPS = [3] * 11 + [1]       # exp group sizes (psum banks per group)
F32 = mybir.dt.float32
FP16 = mybir.dt.float16

# compact-gather runs in slab flat coords (absolute x incl. window offset)
RUNS = {
    0: [(8, 32), (48, 8), (72, 16), (96, 24), (144, 40)],
    1: [(24, 32), (72, 40), (136, 24), (168, 16), (200, 8)],
}
EXP_A = 9.0                          # exp shift (softmax invariant)
DEN_C = 1408.0 * math.exp(-EXP_A)    # dropped-kv constant after shift


def build_nc(debug=False):
    nc = bacc.Bacc()
    dbg = {}
    if debug:
        dbg['qkv'] = nc.dram_tensor("dbg_qkv", [20, H * W], FP16, kind="ExternalOutput")
        dbg['uk0'] = nc.dram_tensor("dbg_uk0", [12, KVP], FP16, kind="ExternalOutput")
        dbg['uv0'] = nc.dram_tensor("dbg_uv0", [12, KVP], FP16, kind="ExternalOutput")
        dbg['opad'] = nc.dram_tensor("dbg_opad", [76, 130 * 50], FP16, kind="ExternalOutput")

    xp_d = nc.dram_tensor("xp", [CIN, 130 * 50], FP16, kind="ExternalInput")
    wA_d = nc.dram_tensor("wA", [128, 3 * 20], FP16, kind="ExternalInput")
    wB_d = nc.dram_tensor("wB", [64, 3 * 20], FP16, kind="ExternalInput")
    bias_d = nc.dram_tensor("bias", [20, 1], F32, kind="ExternalInput")
    wo3_d = nc.dram_tensor("wo3", [76, 3 * 64], FP16, kind="ExternalInput")
    id12_d = nc.dram_tensor("id12", [12, 12], FP16, kind="ExternalInput")
    id128_d = nc.dram_tensor("id128", [128, 128], FP16, kind="ExternalInput")
    # zeroed flange-padded slabs (host-zeroed); device stages interiors
    slab_k = nc.dram_tensor("slab_k", [4, 9216], FP16, kind="ExternalInput")
    slab_v = nc.dram_tensor("slab_v", [4, 9216], FP16, kind="ExternalInput")
    out_d = nc.dram_tensor("out", [COUT, H * W], FP16, kind="ExternalOutput")

    from contextlib import ExitStack

    with tile.TileContext(nc) as tc, ExitStack() as ctx:
        P = ctx.enter_context(tc.tile_pool(name="persist", bufs=1))

        # ---- load constants / x (row-split so conv can start early) ----
        wA_sb = P.tile([128, 3, 20], FP16, tag="wA")
        nc.gpsimd.dma_start(out=wA_sb, in_=wA_d[:, :].rearrange("p (t o) -> p t o", t=3))
        wB_sb = P.tile([64, 3, 20], FP16, tag="wB")
        bias_sb = P.tile([20, 1], F32, tag="bias")
        x2 = P.tile([128, 130, 50], FP16, tag="x2")
        x2f = x2.rearrange("p a b -> p (a b)")
        for i, (r0, r1) in enumerate(
                ((0, 12), (12, 45), (45, 78), (78, 108), (108, 130))):
            nc.sync.dma_start(out=x2f[0:64, 50 * r0: 50 * r1],
                              in_=xp_d[:, 50 * r0: 50 * r1])
            r1b = min(r1, 129)
            nc.gpsimd.dma_start(out=x2f[64:128, 50 * r0: 50 * r1b],
                                in_=xp_d[:, 50 * r0 + 50: 50 * r1b + 50])
            if i == 0:
                nc.gpsimd.dma_start(out=wB_sb, in_=wB_d[:, :].rearrange(
                    "p (t o) -> p t o", t=3))
                nc.gpsimd.dma_start(out=bias_sb, in_=bias_d[:, :])
        wo3_sb = P.tile([76, 3, 64], FP16, tag="wo3")
        nc.sync.dma_start(out=wo3_sb, in_=wo3_d[:, :].rearrange("p (t o) -> p t o", t=3))
        id12 = P.tile([12, 12], FP16, tag="id12")
        nc.sync.dma_start(out=id12, in_=id12_d[:, :])
        id128 = P.tile([128, 128], FP16, tag="id128")
        nc.sync.dma_start(out=id128, in_=id128_d[:, :])

        nega = P.tile([128, 1], F32, tag="nega")
        nc.vector.memset(nega, -EXP_A)
        dummy = P.tile([1, 1], F32, tag="dummy")
        nc.vector.memset(dummy, 0.0)
        # preload the Exp activation table during the prologue shadow
        nc.scalar.activation(dummy, dummy, mybir.ActivationFunctionType.Exp,
                             bias=nega[0:1, 0:1])
        ones128 = P.tile([128, 128], FP16, tag="ones128")
        nc.vector.memset(ones128, 1.0)
        crow = P.tile([128, 4, 13], FP16, tag="crow")
        nc.vector.memset(crow.rearrange("p a b -> p (a b)"), 0.0)
        nc.vector.memset(crow[:, :, 0], DEN_C / 128.0)

        o_pad3 = P.tile([76, 130, 50], FP16, tag="opad3")

        # ---- fused conv q-stack(12)+k(4)+v(4); staging pipelined per chunk
        qkv = P.tile([20, 128, 48], FP16, tag="qkv")
        slk = slab_k[:, :].rearrange("p (a b) -> p a b", a=144)
        slv = slab_v[:, :].rearrange("p (a b) -> p a b", a=144)
        with tc.tile_pool(name="psc", bufs=4, space="PSUM") as psc:
            for chv in range(16):
                ps = psc.tile([20, 8, 48], F32, tag="cps")
                for dx in range(3):
                    nc.tensor.matmul(
                        ps, wA_sb[:, dx, :],
                        x2[:, 8 * chv: 8 * chv + 8, dx: dx + 48],
                        start=(dx == 0), stop=False)
                for dx in range(3):
                    nc.tensor.matmul(
                        ps, wB_sb[:, dx, :],
                        x2[0:64, 8 * chv + 2: 8 * chv + 10, dx: dx + 48],
                        start=False, stop=(dx == 2))
                nc.vector.tensor_scalar_add(
                    qkv[:, 8 * chv: 8 * chv + 8, :], ps, bias_sb[:, 0:1])
                r0 = 8 + 8 * chv
                nc.gpsimd.dma_start(out=slk[:, r0: r0 + 8, 8:56],
                                    in_=qkv[12:16, 8 * chv: 8 * chv + 8, :])
                nc.sync.dma_start(out=slv[:, r0: r0 + 8, 8:56],
                                  in_=qkv[16:20, 8 * chv: 8 * chv + 8, :])

        # keep the PE p-state ramped between conv end and first QK
        with tc.tile_pool(name="warm", bufs=2, space="PSUM") as warm:
            for i in range(29):
                wps = warm.tile([128, 512], F32, tag="w", name="wps")
                nc.tensor.matmul(wps, ones128, x2f[:, 0:512],
                                 start=True, stop=True)

        o_pad3f = o_pad3.rearrange("p a b -> p (a b)")
        for i in range(8):
            nc.vector.memset(o_pad3f[:, 813 * i: min(813 * (i + 1), 6500)], 0.0)

        # ---- islab: 12 rows = flat-shifted aliased views (one DMA each) ----
        SPAN = 6928
        dram = ctx.enter_context(tc.tile_pool(name="dram", bufs=1, space="DRAM"))
        islab_k = dram.tile([12, SPAN], FP16, tag="islab_k")
        islab_v = dram.tile([12, SPAN], FP16, tag="islab_v")
        HSPAN = SPAN // 2
        nc.scalar.dma_start(
            out=islab_v,
            in_=AP(tensor=slab_v, offset=0,
                   ap=[[3072, 3], [6144, 4], [1, SPAN]]))
        nc.gpsimd.dma_start(
            out=islab_k[:, 0:HSPAN],
            in_=AP(tensor=slab_k, offset=0,
                   ap=[[3072, 3], [6144, 4], [1, HSPAN]]))
        nc.sync.dma_start(
            out=islab_k[:, HSPAN:SPAN],
            in_=AP(tensor=slab_k, offset=HSPAN,
                   ap=[[3072, 3], [6144, 4], [1, SPAN - HSPAN]]))

        # ---- direct compact gathers: uk/uv [12, KVP] per window ----
        WP = ctx.enter_context(tc.tile_pool(name="winp", bufs=2))

        HKV = 1152                 # run 0 covers kv chunks 0..8 exactly
        RKV = KVP - HKV

        def gather_run(j, i, islab, dst, eng, pos):
            st, L = RUNS[j][i]
            src = AP(tensor=islab.tensor, offset=islab.offset + st,
                     ap=[[SPAN, 12], [192, NPER], [1, L]])
            eng.dma_start(
                out=dst[:, pos: pos + NPER * L].rearrange(
                    "p (a b) -> p a b", a=NPER),
                in_=src)
            return pos + NPER * L

        def gather_win(j, sched, hi_bufs=()):
            # sched: list of (buf, run_idx, eng); run dst offsets cumulative
            uk_h = WP.tile([12, HKV], FP16, tag="ukh")
            uk_r = WP.tile([12, RKV], FP16, tag="ukr")
            uv_h = WP.tile([12, HKV], FP16, tag="uvh")
            uv_r = WP.tile([12, RKV], FP16, tag="uvr")
            tiles = {("k", 0): uk_h, ("v", 0): uv_h}
            pos = {"k": 0, "v": 0}
            from contextlib import nullcontext
            for (buf, i, eng) in sched:
                islab = islab_k if buf == "k" else islab_v
                cm = tc.high_priority(offset=None) if buf in hi_bufs \
                    else nullcontext()
                with cm:
                    if i == 0:
                        gather_run(j, 0, islab, tiles[buf, 0], eng, 0)
                    else:
                        dst = uk_r if buf == "k" else uv_r
                        pos[buf] = gather_run(j, i, islab, dst, eng,
                                              pos[buf])
            nc.gpsimd.memset(uk_r[:, KVK - HKV: RKV], 0.0)
            nc.gpsimd.memset(uv_r[:, KVK - HKV: RKV], 0.0)
            return (uk_h, uk_r), (uv_h, uv_r)

        uk0, uv0 = gather_win(0, [
            ("k", 0, nc.sync), ("k", 1, nc.gpsimd), ("k", 2, nc.gpsimd),
            ("k", 3, nc.sync), ("k", 4, nc.sync),
            ("v", 0, nc.sync), ("v", 1, nc.gpsimd), ("v", 2, nc.gpsimd),
            ("v", 3, nc.sync), ("v", 4, nc.gpsimd)], hi_bufs=("k",))
        uk1, uv1 = gather_win(1, [
            ("k", i, nc.gpsimd) for i in range(5)] + [
            ("v", i, nc.sync) for i in range(5)])
        uks = (uk0, uk1)
        uvs = (uv0, uv1)

        if debug:
            nc.sync.dma_start(out=dbg['qkv'][:, :], in_=qkv.rearrange("p a b -> p (a b)"))

        # ---- attention ----
        with (
            tc.tile_pool(name="psqk", bufs=1, space="PSUM") as psqk,
            tc.tile_pool(name="psav", bufs=1, space="PSUM") as psav,
            tc.tile_pool(name="exp", bufs=4) as expp,
            tc.tile_pool(name="nrm", bufs=4) as nrm,
        ):
            def make_uvT(j, part):
                uv_h, uv_r = uvs[j]
                n = 9 if part == 0 else NCH - 9
                srcT = uv_h if part == 0 else uv_r
                tr = psqk.tile([128, 512], FP16, tag="tr", bufs=1)
                trv = tr[:, 0:n * 12].rearrange("p (a b) -> p a b", a=n)
                for c in range(n):
                    nc.tensor.matmul(
                        trv[:, c, :], srcT[:, 128 * c: 128 * c + 128], id12,
                        is_transpose=True)
                uvT = WP.tile([128, n, 13], FP16,
                              tag=f"uvT{part}", name=f"uvT{part}")
                nc.vector.tensor_copy(uvT[:, :, 1:13], trv)
                nc.vector.memset(uvT[:, :, 0], 1.0)
                return uvT

            def qk_group(j, qr, qcol, c0, gs):
                qk = psqk.tile([128, 3, 512], F32, tag="qk", bufs=2)
                rhs = qkv[0:12, 64 * qr: 64 * qr + 64,
                          24 * j + 8 * qcol: 24 * j + 8 * qcol + 8]
                uk_h, uk_r = uks[j]
                for b in range(gs):
                    c = c0 + b
                    lhs = (uk_h[:, 128 * c: 128 * c + 128] if c < 9 else
                           uk_r[:, 128 * (c - 9): 128 * (c - 9) + 128])
                    out = qk[:, b, :].rearrange("p (a c) -> p a c", a=64)
                    nc.tensor.matmul(out, lhs, rhs, start=True, stop=True)
                return qk

            def exp_group(qk, gs):
                ex = expp.tile([128, 3, 512], FP16, tag="ex")
                nc.scalar.activation(
                    ex[:, 0:gs, :], qk[:, 0:gs, :],
                    mybir.ActivationFunctionType.Exp, bias=nega[:, 0:1])
                return ex

            def av_group(av, ex, c0, gs, uvTp):
                for b in range(gs):
                    c = c0 + b
                    rhsT = (uvTp[0][:, c, :] if c < 9 else
                            uvTp[1][:, c - 9, :])
                    for m in range(4):
                        nc.tensor.matmul(
                            av[:, m, :],
                            ex[:, b, 128 * m: 128 * m + 128],
                            rhsT,
                            start=False, stop=(c == NCH - 1 and m == 3))

            def finish_qc(av):
                s_sb = nrm.tile([128, 4, 13], F32, tag="s")
                nc.vector.tensor_copy(s_sb, av)
                rec = nrm.tile([128, 4], F32, tag="rec")
                nc.vector.reciprocal(rec, s_sb[:, :, 0])
                o_sb = nrm.tile([128, 4, 12], FP16, tag="o")
                for m in range(4):
                    nc.vector.tensor_scalar_mul(
                        o_sb[:, m, :], s_sb[:, m, 1:13], rec[:, m: m + 1])
                return o_sb

            def emit_otr(j, qr, qcol, o_sb, last=False):
                tr = psqk.tile([128, 512], FP16, tag="tr", bufs=1)
                trv2 = tr[0:12, 0:512].rearrange("p (a b) -> p a b", a=4)
                for m in range(4):
                    nc.tensor.matmul(
                        trv2[:, m, :], o_sb[:, m, :], id128, is_transpose=True)
                blk = tr[0:12, 0:512].rearrange("p (a b) -> p a b", a=64)
                c0 = 1 + 24 * j + 8 * qcol
                for g in range(3):
                    r0 = 1 + 64 * qr - g
                    src = blk
                    rows = 64
                    if r0 < 0:
                        src = blk[:, 1:64, :]
                        rows = 63
                        r0 = 0
                    nc.vector.tensor_copy(
                        o_pad3[32 * g: 32 * g + 12,
                               r0: r0 + rows, c0: c0 + 8], src)

            out_sb = P.tile([COUT, H, W], FP16, tag="outsb")

            # final conv (3-row-stack o_pad3): column-split so most chunks
            # run during attention. A = out cols 0:22 (window-0 only),
            # B = cols 22:48.
            def emit_fconvA(chv):
                tr = psqk.tile([128, 512], FP16, tag="tr", bufs=1, name="trA")
                psA = tr.bitcast(F32)[0:64, 0:176].rearrange(
                    "p (a b) -> p a b", a=8)
                for dx in range(3):
                    nc.tensor.matmul(
                        psA, wo3_sb[:, dx, :],
                        o_pad3[:, 8 * chv: 8 * chv + 8, dx: dx + 22],
                        start=(dx == 0), stop=(dx == 2))
                nc.vector.tensor_copy(
                    out_sb[:, 8 * chv: 8 * chv + 8, 0:22], psA)

            def emit_fconvB(chv, early):
                if early:
                    tr = psqk.tile([128, 512], FP16, tag="tr", bufs=1,
                                   name="trB")
                    psB = tr.bitcast(F32)[0:64, 0:208].rearrange(
                        "p (a b) -> p a b", a=8)
                else:
                    qkt = psqk.tile([128, 3, 512], F32, tag="qk", bufs=2,
                                    name="qkB")
                    psB = qkt[0:64, 0, 0:208].rearrange(
                        "p (a b) -> p a b", a=8)
                for dx in range(3):
                    nc.tensor.matmul(
                        psB, wo3_sb[:, dx, :],
                        o_pad3[:, 8 * chv: 8 * chv + 8, 22 + dx: 48 + dx],
                        start=(dx == 0), stop=(dx == 2))
                if early or chv % 2 == 0:
                    nc.vector.tensor_copy(
                        out_sb[:, 8 * chv: 8 * chv + 8, 22:48], psB)
                else:
                    nc.scalar.activation(
                        out_sb[:, 8 * chv: 8 * chv + 8, 22:48], psB,
                        mybir.ActivationFunctionType.Copy)
                oeng = nc.sync if chv % 2 == 0 else nc.gpsimd
                oeng.dma_start(
                    out=out_d[:, 384 * chv: 384 * (chv + 1)],
                    in_=out_sb[:, 8 * chv: 8 * chv + 8, :].rearrange(
                        "p a b -> p (a b)"))

            def emit_fconvB1n(chv):
                # cols 22:40 of late chunks; ready during the last qc
                tr = psqk.tile([128, 512], FP16, tag="tr", bufs=1,
                               name="trB1")
                ps1 = tr.bitcast(F32)[0:64, 0:136].rearrange(
                    "p (a b) -> p a b", a=8)
                for dx in range(3):
                    nc.tensor.matmul(
                        ps1, wo3_sb[:, dx, :],
                        o_pad3[:, 8 * chv: 8 * chv + 8, 22 + dx: 39 + dx],
                        start=(dx == 0), stop=(dx == 2))
                nc.vector.tensor_copy(
                    out_sb[:, 8 * chv: 8 * chv + 8, 22:39], ps1)

            def emit_fconvB2n(chv):
                # cols 40:48 tail piece + the chunk's out DMA
                qkt = psqk.tile([128, 3, 512], F32, tag="qk", bufs=2,
                                name="qkB2")
                ps2 = qkt[0:64, 0, 0:72].rearrange(
                    "p (a b) -> p a b", a=8)
                for dx in range(3):
                    nc.tensor.matmul(
                        ps2, wo3_sb[:, dx, :],
                        o_pad3[:, 8 * chv: 8 * chv + 8, 39 + dx: 48 + dx],
                        start=(dx == 0), stop=(dx == 2))
                if chv % 2 == 0:
                    nc.vector.tensor_copy(
                        out_sb[:, 8 * chv: 8 * chv + 8, 39:48], ps2)
                else:
                    nc.scalar.activation(
                        out_sb[:, 8 * chv: 8 * chv + 8, 39:48], ps2,
                        mybir.ActivationFunctionType.Copy)
                oeng = nc.sync if chv % 2 == 0 else nc.gpsimd
                oeng.dma_start(
                    out=out_d[:, 384 * chv: 384 * (chv + 1)],
                    in_=out_sb[:, 8 * chv: 8 * chv + 8, :].rearrange(
                        "p a b -> p (a b)"))

            early_q = [("A", c) for c in range(16)] + \
                      [("B", c) for c in range(7)] + \
                      [("B1", c) for c in range(7, 16)]

            uvTs = [[None, None], [None, None]]

            def can_av(j, c0, gs):
                need = 1 if c0 + gs - 1 >= 9 else 0
                return uvTs[j][need] is not None

            for j in range(2):
                pending = None
                for qr in range(2):
                    for qcol in range(3):
                        av = psav.tile([128, 4, 13], F32, tag="av")
                        nc.tensor.matmul(
                            av.rearrange("p a b -> p (a b)"), ones128,
                            crow.rearrange("p a b -> p (a b)"),
                            start=True, stop=False)
                        pend_avs = []
                        c0 = 0
                        for g, gs in enumerate(GRPS):
                            qk = qk_group(j, qr, qcol, c0, gs)
                            if uvTs[j][0] is None and g == 2:
                                uvTs[j][0] = make_uvT(j, 0)
                            if uvTs[j][1] is None and g == 5:
                                uvTs[j][1] = make_uvT(j, 1)
                            if pending is not None:
                                emit_otr(*pending)
                                pending = None
                            elif j == 1 and early_q:
                                kind, c = early_q[0]
                                ok = (kind == "A" or
                                      (kind == "B" and qr == 1) or
                                      (kind == "B1" and qr == 1 and
                                       qcol == 2))
                                if ok:
                                    early_q.pop(0)
                                    if kind == "A":
                                        emit_fconvA(c)
                                    elif kind == "B":
                                        emit_fconvB(c, True)
                                    else:
                                        emit_fconvB1n(c)
                            left = []
                            for (pex, pc0, pgs) in pend_avs:
                                if can_av(j, pc0, pgs):
                                    av_group(av, pex, pc0, pgs, uvTs[j])
                                else:
                                    left.append((pex, pc0, pgs))
                            pend_avs = left
                            ex = exp_group(qk, gs)
                            pend_avs.append((ex, c0, gs))
                            c0 += gs
                        for (pex, pc0, pgs) in pend_avs:
                            av_group(av, pex, pc0, pgs, uvTs[j])
                        o_sb = finish_qc(av)
                        pending = (j, qr, qcol, o_sb)
                if pending is not None:
                    emit_otr(*pending, last=(j == 1))
                    pending = None

            # leftover early chunks (safety) + narrow tail pieces,
            # pipelined on the freed qk slots
            for kind, c in early_q:
                if kind == "A":
                    emit_fconvA(c)
                elif kind == "B":
                    emit_fconvB(c, True)
                else:
                    emit_fconvB1n(c)
            for chv in range(7, 16):
                emit_fconvB2n(chv)

        if debug:
            nc.sync.dma_start(out=dbg['opad'][:, :], in_=o_pad3.rearrange("p a b -> p (a b)"))

    nc.compile()
    return nc


_NC = None


def _get_nc():
    global _NC
    if _NC is None:
        _NC = build_nc()
    return _NC


def make_in_maps(x, wq, bq, wk, bk, wv, bv, wo):
    x = np.asarray(x, np.float32)[0]           # [64, 128, 48]
    xp = np.zeros((CIN, 130, 50), np.float32)
    xp[:, 1:129, 1:49] = x
    xp = xp.reshape(CIN, -1)
    s = np.float32(DPH ** -0.5)

    wq_np = np.asarray(wq, np.float32)
    wk_np = np.asarray(wk, np.float32) * s
    wv_np = np.asarray(wv, np.float32)
    wo_np = np.asarray(wo, np.float32)
    bq_np = np.asarray(bq, np.float32)
    bk_np = np.asarray(bk, np.float32) * s
    bv_np = np.asarray(bv, np.float32)

    zslab = np.zeros((4, 9216), np.float16)
    in_maps = []
    for h in range(8):
        c_lo = (24576 * h) // 9216
        phi = (24576 * h - 9216 * c_lo) // 64
        v_idx = PHIS.index(phi)

        # 20-channel stacked conv weights [O=20, C, 3, 3]
        wstack = np.zeros((20, CIN, 3, 3), np.float32)
        wstack[4 * v_idx: 4 * v_idx + 4] = wq_np[4 * h: 4 * h + 4]
        wstack[12:16] = wk_np[c_lo: c_lo + 4]
        wstack[16:20] = wv_np[c_lo: c_lo + 4]
        bstack = np.zeros((20,), np.float32)
        bstack[4 * v_idx: 4 * v_idx + 4] = bq_np[4 * h: 4 * h + 4]
        bstack[12:16] = bk_np[c_lo: c_lo + 4]
        bstack[16:20] = bv_np[c_lo: c_lo + 4]

        wA = np.zeros((128, 3, 20), np.float32)
        wA[0:64] = np.transpose(wstack[:, :, 0, :], (1, 2, 0))
        wA[64:128] = np.transpose(wstack[:, :, 1, :], (1, 2, 0))
        wB = np.ascontiguousarray(np.transpose(wstack[:, :, 2, :], (1, 2, 0)))

        # wo3 [76, 3(dx), 64]: row 32g + (4v+d) = wo[o, 4h+d, dy=g, dx],
        # nonzero only for the active variant v_idx
        wo3 = np.zeros((76, 3, 64), np.float32)
        for g in range(3):
            wo3[32 * g + 4 * v_idx: 32 * g + 4 * v_idx + 4, :, :] = \
                np.transpose(wo_np[:, 4 * h: 4 * h + 4, g, :], (1, 2, 0))

        m = {
            "xp": xp.astype(np.float16),
            "wA": wA.reshape(128, -1).astype(np.float16),
            "wB": wB.reshape(64, -1).astype(np.float16),
            "bias": bstack.reshape(20, 1),
            "wo3": wo3.reshape(76, -1).astype(np.float16),
            "id12": np.eye(12, dtype=np.float16),
            "id128": np.eye(128, dtype=np.float16),
            "slab_k": zslab,
            "slab_v": zslab,
        }
        in_maps.append(m)
    return in_maps


def kernel(x, wq, bq, wk, bk, wv, bv, wo):
    from concourse.bass_utils import run_bass_kernel_spmd

    nc = _get_nc()
    in_maps = make_in_maps(x, wq, bq, wk, bk, wv, bv, wo)
    res = run_bass_kernel_spmd(nc, in_maps, list(range(8))).results
    out = np.zeros((COUT, H * W), np.float32)
    for m in res:
        out = out + np.asarray(m["out"], np.float32)
    return out.reshape(1, COUT, H, W)
